# revision 45
# baseline (speedup 1.0000x reference)
"""Causal single-head attention (B=8, T=2048, D=1024, HS=64) on 8 TRN2 NeuronCores.

Sharding: data-parallel over batch -- core b computes batch b end-to-end.
No collectives; outputs are post-processed (normalize + transpose) on host.

v4 notes (hard-won from v1-v3 traces):
  - ~6.6us fixed framework preamble; DMA issues ~0.7-1.3us each on the
    issuing queue, transfers contend chip-wide (8 cores share HBM/DMA:
    ~1.3us per 256KB chunk in practice, not the 0.7us single-core number).
  - HAM clock gate: K=8/8 grant needs a ~100%-busy 3.4us window; any later
    window under ~60% busy re-throttles to K=4/8 and re-grant is unreliable
    (v2/v3 ran entire phases 100%-busy at K=4, stuck).  Strategy: one dense
    stream, no phase-boundary bubbles.
  - DMA order: x chunk0 leads the scalar queue, W(QK half) leads the sync
    queue, so the first projection starts ~9us; 4 dep-free bf16 warmup
    matmuls bridge preamble -> first chunk and earn the grant early.
  - Boundary bridges: S^T matmuls of the next attention pair are emitted
    between the V-projection matmuls and the V' transposes (covering the
    DVE vt-add latency); pair1 QK projections are interleaved into pair0's
    attention as PE filler; pair0's final store is deferred behind
    drain_qk(1) so kt pair1 is ready sooner.
  - No device epilogue: numerator^T [64,T] + denominator [1,T] go to DRAM
    in fp32; the host divides + transposes (also better precision).

Per-pair attention (supers a=2p, b=2p+1), k-tiles j < nk = 8|16:
    S^T[k, 0:512]    = K_j @ Q^T_a     (only while j < nka = 4a+4)
    S^T[k, 512:1024] = K_j @ Q^T_b
    P^T = exp(S^T)                     (ScalarE, bf16; logits ~N(0,1))
    causal mask on diagonal tiles      (GPSIMD affine_select, per half)
    ot[65, half] += V'_j^T @ P^T-half  (V' row 64 = ones -> denominator)
"""

import sys

if "/opt/trn_rl_repo" not in sys.path:
    sys.path.insert(0, "/opt/trn_rl_repo")

import os
from contextlib import ExitStack

import numpy as np

import concourse.bass as bass
import concourse.tile as tile
from concourse import bacc, mybir
from concourse.bass_utils import run_bass_kernel_spmd

B, T, D, HS = 8, 2048, 1024, 64
N_CORES = 8
F32 = mybir.dt.float32
BF16 = mybir.dt.bfloat16

TT = 128            # t/k tile (partition dim)
NDT = D // TT       # 8 contraction chunks
NTT = T // TT       # 16 k-tiles
QS = 512            # per-super matmul free dim (PSUM bank limit)
PW = 2 * QS         # pair width (2 supers)
NP = T // PW        # 2 super-pairs
VP = HS + 1         # V' width (64 + ones column)
VPAD = 80           # V' rows padded to a multiple of XBAR_TILE_SRC_ROWS (16)


def build_graph() -> bacc.Bacc:
    nc = bacc.Bacc("TRN2", target_bir_lowering=False, debug=False)

    xt_ext = nc.dram_tensor("xt", [D, T], BF16, kind="ExternalInput").ap()
    # host pre-arranged, split by half for early availability:
    # wqk[p, c*128 + n] = [Wq/8 | Wk][c*128+p, n];  wv likewise [Wv | 0]
    wqk_ext = nc.dram_tensor("wqk", [TT, NDT * TT], BF16,
                             kind="ExternalInput").ap()
    wv_ext = nc.dram_tensor("wv", [TT, NDT * TT], BF16,
                            kind="ExternalInput").ap()
    bcol_ext = nc.dram_tensor("bcol", [TT, 2], F32, kind="ExternalInput").ap()
    # rows 0:64 = (attn @ V)^T numerator, row 64 = softmax denominator
    out_ext = nc.dram_tensor("outT", [VP, T], F32, kind="ExternalOutput").ap()

    with tile.TileContext(nc) as tc, ExitStack() as ctx:
        const = ctx.enter_context(tc.tile_pool(name="const", bufs=1))
        persist = ctx.enter_context(tc.tile_pool(name="persist", bufs=1))
        xt_pool = ctx.enter_context(tc.tile_pool(name="xt", bufs=1))
        vt_pool = ctx.enter_context(tc.tile_pool(name="vt", bufs=2))
        pt_pool = ctx.enter_context(tc.tile_pool(name="pt", bufs=6))
        osb_pool = ctx.enter_context(tc.tile_pool(name="osb", bufs=2))
        warm_pool = ctx.enter_context(tc.tile_pool(name="warm", bufs=1))
        psum = ctx.enter_context(tc.tile_pool(name="ps", bufs=1, space="PSUM"))

        # PSUM (8 banks): tag "proj" bufs=2 x 2 banks holds the live
        # {pqk | pv | ot} set; tag "sp" bufs=2 x 2 banks rotates S^T tiles /
        # V'-transpose temps / warmup.
        def proj_t(name, shape=None):
            return psum.tile(shape or [TT, PW], F32, tag="proj", bufs=2,
                             name=name)

        def sp_t(name, shape=None, dtype=F32):
            return psum.tile(shape or [TT, PW], dtype, tag="sp", bufs=2,
                             name=name)

        # ---- persistent per-core intermediates (bf16 matmul operands) ----
        qt_sb = persist.tile([HS, T], BF16)         # Q^T / 8 (scale folded)
        kt_sb = persist.tile([HS, T], BF16)         # K^T
        # V' [128, 80] per k-tile (cols 0:65 useful; 80-wide slots for the
        # xbar DMA-transpose which writes the full padded block)
        vp_sb = persist.tile([TT, NTT * VPAD], BF16)

        # ---- DMAs.  scalar queue: x pair0 chunks first (first MM input),
        # then the small consts.  sync queue: W halves first, then x pair1.
        xt_sb = [
            xt_pool.tile([TT, PW], BF16, tag=f"xt{c}", bufs=1, name=f"xt{c}_0")
            for c in range(NDT)
        ]
        # pair1 chunks arrive as one strided mega-DMA (single issue + sem);
        # its transfer queues behind W on the sync ring, clear of pair0's.
        xt1_sb = xt_pool.tile([TT, NDT * PW], BF16, tag="xt1", name="xt1_all")
        wqk_sb = const.tile([TT, NDT * TT], BF16)
        wv_sb = const.tile([TT, NDT * TT], BF16)
        bcol_sb = const.tile([TT, 2], F32)

        # pair0 chunks split across both rings for 2x arrival rate; the
        # pair1 mega-DMA goes last so it can't crowd out the early chunks.
        for c in range(0, NDT, 2):
            nc.scalar.dma_start(
                xt_sb[c][:], xt_ext[c * TT:(c + 1) * TT, 0:PW]
            )
        nc.scalar.dma_start(bcol_sb[:], bcol_ext)

        nc.sync.dma_start(wqk_sb[:], wqk_ext)
        nc.sync.dma_start(wv_sb[:], wv_ext)
        for c in range(1, NDT, 2):
            nc.sync.dma_start(
                xt_sb[c][:], xt_ext[c * TT:(c + 1) * TT, 0:PW]
            )
        half_d = NDT // 2 * TT
        nc.sync.dma_start(
            xt1_sb[:, 0:NDT // 2 * PW].rearrange("p (c n) -> p c n", c=NDT // 2),
            xt_ext[0:half_d, PW:2 * PW].rearrange("(c p) n -> p c n", p=TT),
        )
        nc.sync.dma_start(
            xt1_sb[:, NDT // 2 * PW:].rearrange("p (c n) -> p c n", c=NDT // 2),
            xt_ext[half_d:D, PW:2 * PW].rearrange("(c p) n -> p c n", p=TT),
        )

        # ---- HAM kick: dep-free bf16 matmuls bridge preamble -> chunk0 ----
        warm_sb = warm_pool.tile([TT, QS], BF16)
        nc.gpsimd.memset(warm_sb[:], 0.0)
        warm_ps = sp_t("warm_ps", [TT, QS])
        for _ in range(4):
            nc.tensor.matmul(
                warm_ps[:], warm_sb[:, 0:TT], warm_sb[:],
                start=True, stop=True,
            )

        def proj_half(xts, w_sb, pp, chunk_order=None):
            """One half-projection (8 chunk-groups); yields after each chunk
            so it can double as attention PE filler.  chunk_order lets the
            accumulation follow DMA arrival order."""
            order = chunk_order or list(range(NDT))
            for i, c in enumerate(order):
                wsl = w_sb[:, c * TT:(c + 1) * TT]
                for xi in range(2):
                    nc.tensor.matmul(
                        pp[:, xi * QS:(xi + 1) * QS], wsl,
                        xts[c][:, xi * QS:(xi + 1) * QS],
                        start=(i == 0), stop=(i == NDT - 1),
                        skip_group_check=True,
                    )
                if chunk_order is not None and i in (1, 3, 5):
                    # dep-free pad keeps the HAM window dense while the
                    # next chunk's DMA is still in flight
                    nc.tensor.matmul(
                        warm_ps[:], warm_sb[:, 0:TT], warm_sb[:],
                        start=True, stop=True,
                    )
                yield

        def drain_qk(p, pqk):
            nc.vector.tensor_scalar_add(
                qt_sb[:, p * PW:(p + 1) * PW], pqk[0:HS, :], bcol_sb[0:HS, 0:1]
            )
            nc.vector.tensor_scalar_add(
                kt_sb[:, p * PW:(p + 1) * PW], pqk[HS:2 * HS, :],
                bcol_sb[HS:2 * HS, 0:1],
            )

        def vp_finish(p, pv):
            """V bias add (DVE) + V' via ONE blocked xbar DMA-transpose on
            the sync ring -- zero PE cost, single issue slot."""
            vt = vt_pool.tile([VPAD, PW], BF16, tag="vt", name=f"vt{p}")
            nc.gpsimd.memset(vt[HS:VPAD, :], 0.0)
            nc.vector.tensor_scalar_add(vt[0:VP, :], pv[0:VP, :], bcol_sb[0:VP, 1:2])
            nc.sync.dma_start_transpose(
                vp_sb[:, 8 * p * VPAD:(8 * p + 8) * VPAD].rearrange(
                    "q (c n) -> q c n", c=8
                ),
                vt[0:VPAD, :],
            )

        def attn(p, order, pre=None, filler=None):
            """Attention for pair p, k-tiles processed in `order` (narrow
            tiles interleaved among wide ones keeps the PE/exp pipeline
            uniformly PE-bound -- no chain-bound tail windows for the HAM to
            re-throttle on).  `pre` emits this pair's V'-transpose block
            after the first two S^T tiles (bridging the vt-add latency);
            `filler` yields next-pair projection chunks as PE filler.
            Returns deferred store closures for halves not stored inline."""
            a, b = 2 * p, 2 * p + 1
            nka, nk = 4 * a + 4, 4 * b + 4
            a_vis = [i for i, j in enumerate(order) if j < nka]
            a_first, a_last = min(a_vis), max(a_vis)
            ot = proj_t(f"ot{p}", [VP, PW])
            ptiles = {}

            def emit_s(j):
                wide = j < nka
                sp = sp_t(f"sp{p}_{j}")
                ksl = kt_sb[:, j * TT:(j + 1) * TT]
                if wide:
                    nc.tensor.matmul(
                        sp[:, 0:QS], ksl, qt_sb[:, a * QS:(a + 1) * QS],
                        start=True, stop=True,
                    )
                nc.tensor.matmul(
                    sp[:, QS:PW], ksl, qt_sb[:, b * QS:(b + 1) * QS],
                    start=True, stop=True,
                )
                pt = pt_pool.tile([TT, PW], BF16, tag="pt", name=f"pt{p}_{j}")
                if wide:
                    nc.scalar.activation(
                        pt[:], sp[:], mybir.ActivationFunctionType.Exp
                    )
                else:
                    nc.scalar.activation(
                        pt[:, QS:PW], sp[:, QS:PW],
                        mybir.ActivationFunctionType.Exp,
                    )
                for half, s in ((0, a), (1, b)):
                    dd = j - 4 * s
                    if 0 <= dd < 4:
                        nc.gpsimd.affine_select(
                            out=pt[:, half * QS:(half + 1) * QS],
                            in_=pt[:, half * QS:(half + 1) * QS],
                            compare_op=mybir.AluOpType.is_ge,
                            fill=0.0,
                            base=-TT * dd,
                            channel_multiplier=-1,
                            pattern=[[1, QS]],
                        )
                ptiles[j] = pt

            def emit_pv(idx, j):
                pt = ptiles.pop(j)
                vsl = vp_sb[:, j * VPAD:j * VPAD + VP]
                if j < nka:
                    nc.tensor.matmul(
                        ot[:, 0:QS], vsl, pt[:, 0:QS],
                        start=(idx == a_first), stop=(idx == a_last),
                        skip_group_check=True,
                    )
                nc.tensor.matmul(
                    ot[:, QS:PW], vsl, pt[:, QS:PW],
                    start=(idx == 0), stop=(idx == nk - 1),
                    skip_group_check=True,
                )

            def store_half(half):
                osb = osb_pool.tile([VP, QS], F32, tag="osb",
                                    name=f"osb{p}_{half}")
                nc.vector.tensor_copy(osb[:], ot[:, half * QS:(half + 1) * QS])
                nc.sync.dma_start(
                    out_ext[:, (2 * p + half) * QS:(2 * p + half + 1) * QS],
                    osb[:],
                )

            if filler is not None:
                # dep-free PE work ahead of the first S^T weight loads,
                # which gate on the DVE qt/kt drain chain
                next(filler, None)
                next(filler, None)
            if pre is not None:
                # vt-gated transposes go first; the PE reorder window lets
                # the S^T matmuls behind them start as soon as qt/kt land
                pre()
            emit_s(order[0])
            emit_s(order[1])
            deferred = []
            for idx in range(nk):
                if filler is not None:
                    # front-load the filler so it is exhausted before the
                    # exp-gated tail, and its trailing drain runs mid-phase
                    take = 2 if idx < 4 else 1
                    for _ in range(take):
                        next(filler, None)
                if idx + 2 < nk:
                    emit_s(order[idx + 2])
                emit_pv(idx, order[idx])
                if idx == a_last and idx < nk - 1:
                    store_half(0)
            if a_last == nk - 1:
                deferred.append(lambda: store_half(0))
            deferred.append(lambda: store_half(1))
            return deferred

        # ---- schedule: one dense PE stream ----
        # A01 consumes chunks in DMA-arrival order: evens (scalar ring)
        # land before odds (sync ring, queued behind W).
        pqk0 = proj_t("pqk0")
        for _ in proj_half(xt_sb, wqk_sb, pqk0,
                           chunk_order=[0, 2, 4, 1, 6, 3, 5, 7]):
            pass
        drain_qk(0, pqk0)

        pv0 = proj_t("pv0")
        for _ in proj_half(xt_sb, wv_sb, pv0):
            pass

        xt1 = [xt1_sb[:, c * PW:(c + 1) * PW] for c in range(NDT)]

        def filler0():
            pqk1 = proj_t("pqk1")
            yield from proj_half(xt1, wqk_sb, pqk1)
            drain_qk(1, pqk1)   # on DVE mid-attention, off the corridor
            yield

        def pre0():
            # dep-free pad while the qt/kt/vt adds drain on DVE
            for _ in range(4):
                nc.tensor.matmul(
                    warm_ps[:], warm_sb[:, 0:TT], warm_sb[:],
                    start=True, stop=True,
                )
            vp_finish(0, pv0)

        # pair0: every k-tile is diagonal in one half; alternate wide/narrow
        # so the stream stays PE-bound end to end.
        stores0 = attn(
            0,
            order=[0, 4, 1, 5, 2, 6, 3, 7],
            pre=pre0,
            filler=filler0(),
        )
        for s in stores0:   # overlapped by attn1's first S^T matmuls
            s()

        def filler1():
            """pair1 V projection as attn1 PE filler; V' finishes off-PE
            (DVE bias add + sync-ring DMA-transposes)."""
            pv1 = proj_t("pv1")
            yield from proj_half(xt1, wv_sb, pv1)
            vp_finish(1, pv1)
            yield

        # pair1: first tiles use pair0's V' (ready) while the filler builds
        # pair1's; a-half finishes at tile 11 (pos 14) so its store overlaps
        # the final narrow tile.
        stores1 = attn(
            1,
            order=[0, 1, 2, 3, 4, 5, 6, 7, 12, 8, 13, 9, 14, 10, 11, 15],
            filler=filler1(),
        )
        for s in stores1:
            s()

    nc.compile()
    return nc


def make_inputs(x_b, Wq, bq, Wk, bk, Wv, bv):
    """Host-side prep for one core's in_map (x_b: [T, D] fp32)."""
    import ml_dtypes

    bf = ml_dtypes.bfloat16
    scale = 1.0 / np.sqrt(np.float32(HS))
    wqk = np.zeros((D, TT), dtype=np.float32)
    wqk[:, 0:HS] = Wq * scale
    wqk[:, HS:2 * HS] = Wk
    wv = np.zeros((D, TT), dtype=np.float32)
    wv[:, 0:HS] = Wv

    def chunk_major(w):
        # w2[p, c*128 + n] = w[c*128 + p, n] -> contiguous [128, 1024] DMA
        return np.ascontiguousarray(
            w.reshape(NDT, TT, TT).transpose(1, 0, 2).reshape(TT, NDT * TT)
        )

    bcol = np.zeros((TT, 2), dtype=np.float32)
    bcol[0:HS, 0] = bq * scale
    bcol[HS:2 * HS, 0] = bk
    bcol[0:HS, 1] = bv
    bcol[HS, 1] = 1.0
    return {
        "xt": np.ascontiguousarray(x_b.T).astype(bf),
        "wqk": chunk_major(wqk).astype(bf),
        "wv": chunk_major(wv).astype(bf),
        "bcol": bcol,
    }


def finish_output(outT):
    """Host-side normalize + transpose: outT [65, T] -> [T, HS]."""
    o = np.asarray(outT, dtype=np.float32)
    return (o[0:HS, :] / o[HS:HS + 1, :]).T


_NC_CACHE = None


def _get_nc():
    global _NC_CACHE
    if _NC_CACHE is None:
        _NC_CACHE = build_graph()
    return _NC_CACHE


def kernel(x, Wq, bq, Wk, bk, Wv, bv):
    x = np.asarray(x, dtype=np.float32)
    args = [np.asarray(a, dtype=np.float32) for a in (Wq, bq, Wk, bk, Wv, bv)]
    nc = _get_nc()
    in_maps = [make_inputs(x[b], *args) for b in range(N_CORES)]
    trace = os.environ.get("BASS_ATTN_TRACE", "0") == "1"
    res = run_bass_kernel_spmd(
        nc, in_maps, core_ids=list(range(N_CORES)), trace=trace
    )
    if trace:
        print(
            f"HW exec time: {res.exec_time_ns} ns "
            f"(mean {res.mean_exec_time_ns}, max core {res.max_exec_time_core_id})"
        )
    out = np.stack(
        [finish_output(res.results[b]["outT"]) for b in range(N_CORES)], axis=0
    )
    return out


# revision 46
# speedup vs baseline: 1.0782x; 1.0782x over previous
"""Causal single-head attention (B=8, T=2048, D=1024, HS=64) on 8 TRN2 NeuronCores.

Sharding: data-parallel over batch -- core b computes batch b end-to-end.
No collectives; outputs are post-processed (normalize + transpose) on host.

v4 notes (hard-won from v1-v3 traces):
  - ~6.6us fixed framework preamble; DMA issues ~0.7-1.3us each on the
    issuing queue, transfers contend chip-wide (8 cores share HBM/DMA:
    ~1.3us per 256KB chunk in practice, not the 0.7us single-core number).
  - HAM clock gate: K=8/8 grant needs a ~100%-busy 3.4us window; any later
    window under ~60% busy re-throttles to K=4/8 and re-grant is unreliable
    (v2/v3 ran entire phases 100%-busy at K=4, stuck).  Strategy: one dense
    stream, no phase-boundary bubbles.
  - DMA order: x chunk0 leads the scalar queue, W(QK half) leads the sync
    queue, so the first projection starts ~9us; 4 dep-free bf16 warmup
    matmuls bridge preamble -> first chunk and earn the grant early.
  - Boundary bridges: S^T matmuls of the next attention pair are emitted
    between the V-projection matmuls and the V' transposes (covering the
    DVE vt-add latency); pair1 QK projections are interleaved into pair0's
    attention as PE filler; pair0's final store is deferred behind
    drain_qk(1) so kt pair1 is ready sooner.
  - No device epilogue: numerator^T [64,T] + denominator [1,T] go to DRAM
    in fp32; the host divides + transposes (also better precision).

Per-pair attention (supers a=2p, b=2p+1), k-tiles j < nk = 8|16:
    S^T[k, 0:512]    = K_j @ Q^T_a     (only while j < nka = 4a+4)
    S^T[k, 512:1024] = K_j @ Q^T_b
    P^T = exp(S^T)                     (ScalarE, bf16; logits ~N(0,1))
    causal mask on diagonal tiles      (GPSIMD affine_select, per half)
    ot[65, half] += V'_j^T @ P^T-half  (V' row 64 = ones -> denominator)
"""

import sys

if "/opt/trn_rl_repo" not in sys.path:
    sys.path.insert(0, "/opt/trn_rl_repo")

import os
from contextlib import ExitStack

import numpy as np

import concourse.bass as bass
import concourse.tile as tile
from concourse import bacc, mybir
from concourse.bass_utils import run_bass_kernel_spmd

B, T, D, HS = 8, 2048, 1024, 64
N_CORES = 8
F32 = mybir.dt.float32
BF16 = mybir.dt.bfloat16

TT = 128            # t/k tile (partition dim)
NDT = D // TT       # 8 contraction chunks
NTT = T // TT       # 16 k-tiles
QS = 512            # per-super matmul free dim (PSUM bank limit)
PW = 2 * QS         # pair width (2 supers)
NP = T // PW        # 2 super-pairs
VP = HS + 1         # V' width (64 + ones column)
VPAD = 80           # V' rows padded to a multiple of XBAR_TILE_SRC_ROWS (16)


def build_graph() -> bacc.Bacc:
    nc = bacc.Bacc("TRN2", target_bir_lowering=False, debug=False)

    xt_ext = nc.dram_tensor("xt", [D, T], BF16, kind="ExternalInput").ap()
    # host pre-arranged, split by half for early availability:
    # wqk[p, c*128 + n] = [Wq/8 | Wk][c*128+p, n];  wv likewise [Wv | 0]
    wqk_ext = nc.dram_tensor("wqk", [TT, NDT * TT], BF16,
                             kind="ExternalInput").ap()
    wv_ext = nc.dram_tensor("wv", [TT, NDT * TT], BF16,
                            kind="ExternalInput").ap()
    bcol_ext = nc.dram_tensor("bcol", [TT, 2], F32, kind="ExternalInput").ap()
    # rows 0:64 = (attn @ V)^T numerator, row 64 = softmax denominator
    out_ext = nc.dram_tensor("outT", [VP, T], F32, kind="ExternalOutput").ap()

    with tile.TileContext(nc) as tc, ExitStack() as ctx:
        const = ctx.enter_context(tc.tile_pool(name="const", bufs=1))
        persist = ctx.enter_context(tc.tile_pool(name="persist", bufs=1))
        xt_pool = ctx.enter_context(tc.tile_pool(name="xt", bufs=1))
        vt_pool = ctx.enter_context(tc.tile_pool(name="vt", bufs=2))
        pt_pool = ctx.enter_context(tc.tile_pool(name="pt", bufs=4))
        osb_pool = ctx.enter_context(tc.tile_pool(name="osb", bufs=2))
        warm_pool = ctx.enter_context(tc.tile_pool(name="warm", bufs=1))
        psum = ctx.enter_context(tc.tile_pool(name="ps", bufs=1, space="PSUM"))

        # PSUM (8 banks): tag "proj" bufs=2 x 2 banks holds the live
        # {pqk | pv | ot} set; tag "sp" bufs=2 x 2 banks rotates S^T tiles /
        # V'-transpose temps / warmup.
        def proj_t(name, shape=None):
            return psum.tile(shape or [TT, PW], F32, tag="proj", bufs=2,
                             name=name)

        def sp_t(name, shape=None, dtype=F32):
            return psum.tile(shape or [TT, PW], dtype, tag="sp", bufs=2,
                             name=name)

        # ---- persistent per-core intermediates (bf16 matmul operands) ----
        qt_sb = persist.tile([HS, T], BF16)         # Q^T / 8 (scale folded)
        kt_sb = persist.tile([HS, T], BF16)         # K^T
        # V' [128, 80] per k-tile (cols 0:65 useful; 80-wide slots for the
        # xbar DMA-transpose which writes the full padded block)
        vp_sb = persist.tile([TT, NTT * VPAD], BF16)

        # ---- DMAs.  scalar queue: x pair0 chunks first (first MM input),
        # then the small consts.  sync queue: W halves first, then x pair1.
        xt_sb = [
            xt_pool.tile([TT, PW], BF16, tag=f"xt{c}", bufs=1, name=f"xt{c}_0")
            for c in range(NDT)
        ]
        # pair1 chunks arrive as one strided mega-DMA (single issue + sem);
        # its transfer queues behind W on the sync ring, clear of pair0's.
        xt1_sb = xt_pool.tile([TT, NDT * PW], BF16, tag="xt1", name="xt1_all")
        wqk_sb = const.tile([TT, NDT * TT], BF16)
        wv_sb = const.tile([TT, NDT * TT], BF16)
        bcol_sb = const.tile([TT, 2], F32)

        # pair0 chunks split across both rings for 2x arrival rate; the
        # pair1 mega-DMA goes last so it can't crowd out the early chunks.
        for c in range(0, NDT, 2):
            nc.scalar.dma_start(
                xt_sb[c][:], xt_ext[c * TT:(c + 1) * TT, 0:PW]
            )
        nc.scalar.dma_start(bcol_sb[:], bcol_ext)

        nc.sync.dma_start(wqk_sb[:], wqk_ext)
        nc.sync.dma_start(wv_sb[:], wv_ext)
        for c in range(1, NDT, 2):
            nc.sync.dma_start(
                xt_sb[c][:], xt_ext[c * TT:(c + 1) * TT, 0:PW]
            )
        half_d = NDT // 2 * TT
        nc.sync.dma_start(
            xt1_sb[:, 0:NDT // 2 * PW].rearrange("p (c n) -> p c n", c=NDT // 2),
            xt_ext[0:half_d, PW:2 * PW].rearrange("(c p) n -> p c n", p=TT),
        )
        nc.sync.dma_start(
            xt1_sb[:, NDT // 2 * PW:].rearrange("p (c n) -> p c n", c=NDT // 2),
            xt_ext[half_d:D, PW:2 * PW].rearrange("(c p) n -> p c n", p=TT),
        )

        # ---- HAM kick: dep-free bf16 matmuls bridge preamble -> chunk0 ----
        warm_sb = warm_pool.tile([TT, QS], BF16)
        nc.gpsimd.memset(warm_sb[:], 0.0)
        warm_ps = sp_t("warm_ps", [TT, QS])
        for _ in range(4):
            nc.tensor.matmul(
                warm_ps[:], warm_sb[:, 0:TT], warm_sb[:],
                start=True, stop=True,
            )

        def proj_half(xts, w_sb, pp, chunk_order=None):
            """One half-projection (8 chunk-groups); yields after each chunk
            so it can double as attention PE filler.  chunk_order lets the
            accumulation follow DMA arrival order."""
            order = chunk_order or list(range(NDT))
            for i, c in enumerate(order):
                wsl = w_sb[:, c * TT:(c + 1) * TT]
                for xi in range(2):
                    nc.tensor.matmul(
                        pp[:, xi * QS:(xi + 1) * QS], wsl,
                        xts[c][:, xi * QS:(xi + 1) * QS],
                        start=(i == 0), stop=(i == NDT - 1),
                        skip_group_check=True,
                    )
                if chunk_order is not None and i in (1, 3, 5):
                    # dep-free pad keeps the HAM window dense while the
                    # next chunk's DMA is still in flight
                    nc.tensor.matmul(
                        warm_ps[:], warm_sb[:, 0:TT], warm_sb[:],
                        start=True, stop=True,
                    )
                yield

        def drain_qk(p, pqk):
            nc.vector.tensor_scalar_add(
                qt_sb[:, p * PW:(p + 1) * PW], pqk[0:HS, :], bcol_sb[0:HS, 0:1]
            )
            nc.vector.tensor_scalar_add(
                kt_sb[:, p * PW:(p + 1) * PW], pqk[HS:2 * HS, :],
                bcol_sb[HS:2 * HS, 0:1],
            )

        def vp_finish(p, pv):
            """V bias add (DVE) + V' via ONE blocked xbar DMA-transpose on
            the sync ring -- zero PE cost, single issue slot."""
            vt = vt_pool.tile([VPAD, PW], BF16, tag="vt", name=f"vt{p}")
            nc.gpsimd.memset(vt[HS:VPAD, :], 0.0)
            nc.vector.tensor_scalar_add(vt[0:VP, :], pv[0:VP, :], bcol_sb[0:VP, 1:2])
            nc.sync.dma_start_transpose(
                vp_sb[:, 8 * p * VPAD:(8 * p + 8) * VPAD].rearrange(
                    "q (c n) -> q c n", c=8
                ),
                vt[0:VPAD, :],
            )

        def attn(p, order, pre=None, filler=None):
            """Attention for pair p, k-tiles processed in `order` (narrow
            tiles interleaved among wide ones keeps the PE/exp pipeline
            uniformly PE-bound -- no chain-bound tail windows for the HAM to
            re-throttle on).  `pre` emits this pair's V'-transpose block
            after the first two S^T tiles (bridging the vt-add latency);
            `filler` yields next-pair projection chunks as PE filler.
            Returns deferred store closures for halves not stored inline."""
            a, b = 2 * p, 2 * p + 1
            nka, nk = 4 * a + 4, 4 * b + 4
            a_vis = [i for i, j in enumerate(order) if j < nka]
            a_first, a_last = min(a_vis), max(a_vis)
            ot = proj_t(f"ot{p}", [VP, PW])
            ptiles = {}

            def emit_s(j):
                wide = j < nka
                sp = sp_t(f"sp{p}_{j}")
                ksl = kt_sb[:, j * TT:(j + 1) * TT]
                if wide:
                    nc.tensor.matmul(
                        sp[:, 0:QS], ksl, qt_sb[:, a * QS:(a + 1) * QS],
                        start=True, stop=True,
                    )
                nc.tensor.matmul(
                    sp[:, QS:PW], ksl, qt_sb[:, b * QS:(b + 1) * QS],
                    start=True, stop=True,
                )
                pt = pt_pool.tile([TT, PW], BF16, tag="pt", name=f"pt{p}_{j}")
                if wide:
                    nc.scalar.activation(
                        pt[:], sp[:], mybir.ActivationFunctionType.Exp
                    )
                else:
                    nc.scalar.activation(
                        pt[:, QS:PW], sp[:, QS:PW],
                        mybir.ActivationFunctionType.Exp,
                    )
                for half, s in ((0, a), (1, b)):
                    dd = j - 4 * s
                    if 0 <= dd < 4:
                        nc.gpsimd.affine_select(
                            out=pt[:, half * QS:(half + 1) * QS],
                            in_=pt[:, half * QS:(half + 1) * QS],
                            compare_op=mybir.AluOpType.is_ge,
                            fill=0.0,
                            base=-TT * dd,
                            channel_multiplier=-1,
                            pattern=[[1, QS]],
                        )
                ptiles[j] = pt

            def emit_pv(idx, j):
                pt = ptiles.pop(j)
                vsl = vp_sb[:, j * VPAD:j * VPAD + VP]
                if j < nka:
                    nc.tensor.matmul(
                        ot[:, 0:QS], vsl, pt[:, 0:QS],
                        start=(idx == a_first), stop=(idx == a_last),
                        skip_group_check=True,
                    )
                nc.tensor.matmul(
                    ot[:, QS:PW], vsl, pt[:, QS:PW],
                    start=(idx == 0), stop=(idx == nk - 1),
                    skip_group_check=True,
                )

            def store_half(half):
                osb = osb_pool.tile([VP, QS], F32, tag="osb",
                                    name=f"osb{p}_{half}")
                nc.vector.tensor_copy(osb[:], ot[:, half * QS:(half + 1) * QS])
                nc.sync.dma_start(
                    out_ext[:, (2 * p + half) * QS:(2 * p + half + 1) * QS],
                    osb[:],
                )

            if filler is not None:
                # dep-free PE work ahead of the first S^T weight loads,
                # which gate on the DVE qt/kt drain chain
                next(filler, None)
                next(filler, None)
            if pre is not None:
                # vt-gated transposes go first; the PE reorder window lets
                # the S^T matmuls behind them start as soon as qt/kt land
                pre()
            emit_s(order[0])
            emit_s(order[1])
            deferred = []
            for idx in range(nk):
                if filler is not None:
                    # front-load the filler so it is exhausted before the
                    # exp-gated tail, and its trailing drain runs mid-phase
                    take = 2 if idx < 4 else 1
                    for _ in range(take):
                        next(filler, None)
                if idx + 2 < nk:
                    emit_s(order[idx + 2])
                emit_pv(idx, order[idx])
                if idx == a_last and idx < nk - 1:
                    store_half(0)
            if a_last == nk - 1:
                deferred.append(lambda: store_half(0))
            deferred.append(lambda: store_half(1))
            return deferred

        # ---- schedule: one dense PE stream ----
        # A01 consumes chunks in DMA-arrival order: evens (scalar ring)
        # land before odds (sync ring, queued behind W).
        pqk0 = proj_t("pqk0")
        for _ in proj_half(xt_sb, wqk_sb, pqk0,
                           chunk_order=[0, 2, 4, 1, 6, 3, 5, 7]):
            pass
        drain_qk(0, pqk0)

        pv0 = proj_t("pv0")
        for _ in proj_half(xt_sb, wv_sb, pv0):
            pass

        xt1 = [xt1_sb[:, c * PW:(c + 1) * PW] for c in range(NDT)]

        def filler0():
            pqk1 = proj_t("pqk1")
            yield from proj_half(xt1, wqk_sb, pqk1)
            drain_qk(1, pqk1)   # on DVE mid-attention, off the corridor
            yield

        def pre0():
            # dep-free pad while the vt-add drains on DVE
            for _ in range(2):
                nc.tensor.matmul(
                    warm_ps[:], warm_sb[:, 0:TT], warm_sb[:],
                    start=True, stop=True,
                )
            vp_finish(0, pv0)

        # pair0: every k-tile is diagonal in one half; alternate wide/narrow
        # so the stream stays PE-bound end to end.
        stores0 = attn(
            0,
            order=[0, 4, 1, 5, 2, 6, 3, 7],
            pre=pre0,
            filler=filler0(),
        )
        for s in stores0:   # overlapped by attn1's first S^T matmuls
            s()

        def filler1():
            """pair1 V projection as attn1 PE filler; V' finishes off-PE
            (DVE bias add + sync-ring DMA-transposes)."""
            pv1 = proj_t("pv1")
            yield from proj_half(xt1, wv_sb, pv1)
            vp_finish(1, pv1)
            yield

        # pair1: first tiles use pair0's V' (ready) while the filler builds
        # pair1's; a-half finishes at tile 11 (pos 14) so its store overlaps
        # the final narrow tile.
        stores1 = attn(
            1,
            order=[0, 1, 2, 3, 4, 5, 6, 7, 12, 8, 13, 9, 14, 10, 11, 15],
            filler=filler1(),
        )
        for s in stores1:
            s()

    nc.compile()
    return nc


def make_inputs(x_b, Wq, bq, Wk, bk, Wv, bv):
    """Host-side prep for one core's in_map (x_b: [T, D] fp32)."""
    import ml_dtypes

    bf = ml_dtypes.bfloat16
    scale = 1.0 / np.sqrt(np.float32(HS))
    wqk = np.zeros((D, TT), dtype=np.float32)
    wqk[:, 0:HS] = Wq * scale
    wqk[:, HS:2 * HS] = Wk
    wv = np.zeros((D, TT), dtype=np.float32)
    wv[:, 0:HS] = Wv

    def chunk_major(w):
        # w2[p, c*128 + n] = w[c*128 + p, n] -> contiguous [128, 1024] DMA
        return np.ascontiguousarray(
            w.reshape(NDT, TT, TT).transpose(1, 0, 2).reshape(TT, NDT * TT)
        )

    bcol = np.zeros((TT, 2), dtype=np.float32)
    bcol[0:HS, 0] = bq * scale
    bcol[HS:2 * HS, 0] = bk
    bcol[0:HS, 1] = bv
    bcol[HS, 1] = 1.0
    return {
        "xt": np.ascontiguousarray(x_b.T).astype(bf),
        "wqk": chunk_major(wqk).astype(bf),
        "wv": chunk_major(wv).astype(bf),
        "bcol": bcol,
    }


def finish_output(outT):
    """Host-side normalize + transpose: outT [65, T] -> [T, HS]."""
    o = np.asarray(outT, dtype=np.float32)
    return (o[0:HS, :] / o[HS:HS + 1, :]).T


_NC_CACHE = None


def _get_nc():
    global _NC_CACHE
    if _NC_CACHE is None:
        _NC_CACHE = build_graph()
    return _NC_CACHE


def kernel(x, Wq, bq, Wk, bk, Wv, bv):
    x = np.asarray(x, dtype=np.float32)
    args = [np.asarray(a, dtype=np.float32) for a in (Wq, bq, Wk, bk, Wv, bv)]
    nc = _get_nc()
    in_maps = [make_inputs(x[b], *args) for b in range(N_CORES)]
    trace = os.environ.get("BASS_ATTN_TRACE", "0") == "1"
    res = run_bass_kernel_spmd(
        nc, in_maps, core_ids=list(range(N_CORES)), trace=trace
    )
    if trace:
        print(
            f"HW exec time: {res.exec_time_ns} ns "
            f"(mean {res.mean_exec_time_ns}, max core {res.max_exec_time_core_id})"
        )
    out = np.stack(
        [finish_output(res.results[b]["outT"]) for b in range(N_CORES)], axis=0
    )
    return out


# revision 47
# speedup vs baseline: 1.0876x; 1.0087x over previous
"""Causal single-head attention (B=8, T=2048, D=1024, HS=64) on 8 TRN2 NeuronCores.

Sharding: data-parallel over batch -- core b computes batch b end-to-end.
No collectives; outputs are post-processed (normalize + transpose) on host.

Design notes (from v1-v5 trace analysis):
  - ~6.6us fixed framework preamble; DMA issues ~0.7-1.3us each on the
    issuing queue; transfers contend chip-wide (8 cores share HBM: ~1.3us
    per 256KB chunk in practice).
  - HAM clock gate: K=8/8 grant needs a ~100%-busy 3.4us window; any later
    window under ~60% busy re-throttles to K=4/8 (half clock) and re-grant
    is unreliable.  Everything is scheduled as ONE dense PE stream with
    dep-free pads over unavoidable gaps.
  - DMA: x pair0 chunks split across both HWDGE rings (evens on scalar,
    odds on sync behind the W halves), consumed in arrival order; pair1
    arrives as two strided mega-DMAs behind them.  W is host-pre-arranged
    chunk-major so its DMA is contiguous.
  - PE stream: warmup pads -> QK-proj pair0 -> V-proj pair0 -> attn pair0
    (with QK-proj pair1 as per-k-tile PE filler, qt/kt drain mid-phase on
    DVE) -> attn pair1 (with V-proj pair1 as filler).  Narrow (single-
    super) k-tiles are interleaved among wide ones so the exp pipeline
    never leaves the PE idle at a phase tail.
  - V' ([k,h] layout + ones row for the denominator) is produced by ONE
    blocked xbar DMA-transpose per pair on the idle sync ring (padded to
    80 rows; 16-row tile granularity) -- zero PE/DVE transpose cost.
  - No device epilogue: numerator^T [64,T] + denominator [1,T] stream out
    per half as soon as their accumulation stops; the host divides and
    transposes in fp32.

Per-pair attention (supers a=2p, b=2p+1), k-tiles j < nk = 8|16:
    S^T[k, 0:512]    = K_j @ Q^T_a     (only while j < nka = 4a+4)
    S^T[k, 512:1024] = K_j @ Q^T_b
    P^T = exp(S^T)                     (ScalarE, bf16; logits ~N(0,1))
    causal mask on diagonal tiles      (GPSIMD affine_select, per half)
    ot[65, half] += V'_j^T @ P^T-half  (V' row 64 = ones -> denominator)
"""

import sys

if "/opt/trn_rl_repo" not in sys.path:
    sys.path.insert(0, "/opt/trn_rl_repo")

import os
from contextlib import ExitStack

import numpy as np

import concourse.bass as bass
import concourse.tile as tile
from concourse import bacc, mybir
from concourse.bass_utils import run_bass_kernel_spmd

B, T, D, HS = 8, 2048, 1024, 64
N_CORES = 8
F32 = mybir.dt.float32
BF16 = mybir.dt.bfloat16

TT = 128            # t/k tile (partition dim)
NDT = D // TT       # 8 contraction chunks
NTT = T // TT       # 16 k-tiles
QS = 512            # per-super matmul free dim (PSUM bank limit)
PW = 2 * QS         # pair width (2 supers)
NP = T // PW        # 2 super-pairs
VP = HS + 1         # V' width (64 + ones column)
VPAD = 80           # V' rows padded to a multiple of XBAR_TILE_SRC_ROWS (16)


def build_graph() -> bacc.Bacc:
    nc = bacc.Bacc("TRN2", target_bir_lowering=False, debug=False)

    xt_ext = nc.dram_tensor("xt", [D, T], BF16, kind="ExternalInput").ap()
    # host pre-arranged, split by half for early availability:
    # wqk[p, c*128 + n] = [Wq/8 | Wk][c*128+p, n];  wv likewise [Wv | 0]
    wqk_ext = nc.dram_tensor("wqk", [TT, NDT * TT], BF16,
                             kind="ExternalInput").ap()
    wv_ext = nc.dram_tensor("wv", [TT, NDT * TT], BF16,
                            kind="ExternalInput").ap()
    bcol_ext = nc.dram_tensor("bcol", [TT, 2], F32, kind="ExternalInput").ap()
    # rows 0:64 = (attn @ V)^T numerator, row 64 = softmax denominator
    out_ext = nc.dram_tensor("outT", [VP, T], F32, kind="ExternalOutput").ap()

    with tile.TileContext(nc) as tc, ExitStack() as ctx:
        const = ctx.enter_context(tc.tile_pool(name="const", bufs=1))
        persist = ctx.enter_context(tc.tile_pool(name="persist", bufs=1))
        xt_pool = ctx.enter_context(tc.tile_pool(name="xt", bufs=1))
        vt_pool = ctx.enter_context(tc.tile_pool(name="vt", bufs=2))
        pt_pool = ctx.enter_context(tc.tile_pool(name="pt", bufs=4))
        osb_pool = ctx.enter_context(tc.tile_pool(name="osb", bufs=2))
        warm_pool = ctx.enter_context(tc.tile_pool(name="warm", bufs=1))
        psum = ctx.enter_context(tc.tile_pool(name="ps", bufs=1, space="PSUM"))

        # PSUM (8 banks): tag "proj" bufs=2 x 2 banks holds the live
        # {pqk | pv | ot} set; tag "sp" bufs=2 x 2 banks rotates S^T tiles /
        # V'-transpose temps / warmup.
        def proj_t(name, shape=None):
            return psum.tile(shape or [TT, PW], F32, tag="proj", bufs=2,
                             name=name)

        def sp_t(name, shape=None, dtype=F32):
            return psum.tile(shape or [TT, PW], dtype, tag="sp", bufs=2,
                             name=name)

        # ---- persistent per-core intermediates (bf16 matmul operands) ----
        qt_sb = persist.tile([HS, T], BF16)         # Q^T / 8 (scale folded)
        kt_sb = persist.tile([HS, T], BF16)         # K^T
        # V' [128, 80] per k-tile (cols 0:65 useful; 80-wide slots for the
        # xbar DMA-transpose which writes the full padded block)
        vp_sb = persist.tile([TT, NTT * VPAD], BF16)

        # ---- DMAs.  scalar queue: x pair0 chunks first (first MM input),
        # then the small consts.  sync queue: W halves first, then x pair1.
        xt_sb = [
            xt_pool.tile([TT, PW], BF16, tag=f"xt{c}", bufs=1, name=f"xt{c}_0")
            for c in range(NDT)
        ]
        # pair1 chunks arrive as one strided mega-DMA (single issue + sem);
        # its transfer queues behind W on the sync ring, clear of pair0's.
        xt1_sb = xt_pool.tile([TT, NDT * PW], BF16, tag="xt1", name="xt1_all")
        wqk_sb = const.tile([TT, NDT * TT], BF16)
        wv_sb = const.tile([TT, NDT * TT], BF16)
        bcol_sb = const.tile([TT, 2], F32)

        # pair0 chunks split across both rings for 2x arrival rate; the
        # pair1 mega-DMA goes last so it can't crowd out the early chunks.
        for c in range(0, NDT, 2):
            nc.scalar.dma_start(
                xt_sb[c][:], xt_ext[c * TT:(c + 1) * TT, 0:PW]
            )
        nc.scalar.dma_start(bcol_sb[:], bcol_ext)

        nc.sync.dma_start(wqk_sb[:], wqk_ext)
        nc.sync.dma_start(wv_sb[:], wv_ext)
        for c in range(1, NDT, 2):
            nc.sync.dma_start(
                xt_sb[c][:], xt_ext[c * TT:(c + 1) * TT, 0:PW]
            )
        half_d = NDT // 2 * TT
        nc.sync.dma_start(
            xt1_sb[:, 0:NDT // 2 * PW].rearrange("p (c n) -> p c n", c=NDT // 2),
            xt_ext[0:half_d, PW:2 * PW].rearrange("(c p) n -> p c n", p=TT),
        )
        nc.sync.dma_start(
            xt1_sb[:, NDT // 2 * PW:].rearrange("p (c n) -> p c n", c=NDT // 2),
            xt_ext[half_d:D, PW:2 * PW].rearrange("(c p) n -> p c n", p=TT),
        )

        # ---- HAM kick: dep-free bf16 matmuls bridge preamble -> chunk0 ----
        warm_sb = warm_pool.tile([TT, QS], BF16)
        nc.gpsimd.memset(warm_sb[:], 0.0)
        warm_ps = sp_t("warm_ps", [TT, QS])
        for _ in range(4):
            nc.tensor.matmul(
                warm_ps[:], warm_sb[:, 0:TT], warm_sb[:],
                start=True, stop=True,
            )

        def proj_half(xts, w_sb, pp, chunk_order=None):
            """One half-projection (8 chunk-groups); yields after each chunk
            so it can double as attention PE filler.  chunk_order lets the
            accumulation follow DMA arrival order."""
            order = chunk_order or list(range(NDT))
            for i, c in enumerate(order):
                wsl = w_sb[:, c * TT:(c + 1) * TT]
                for xi in range(2):
                    nc.tensor.matmul(
                        pp[:, xi * QS:(xi + 1) * QS], wsl,
                        xts[c][:, xi * QS:(xi + 1) * QS],
                        start=(i == 0), stop=(i == NDT - 1),
                        skip_group_check=True,
                    )
                if chunk_order is not None and i in (1, 3, 5):
                    # dep-free pad keeps the HAM window dense while the
                    # next chunk's DMA is still in flight
                    nc.tensor.matmul(
                        warm_ps[:], warm_sb[:, 0:TT], warm_sb[:],
                        start=True, stop=True,
                    )
                yield

        def drain_qk(p, pqk):
            nc.vector.tensor_scalar_add(
                qt_sb[:, p * PW:(p + 1) * PW], pqk[0:HS, :], bcol_sb[0:HS, 0:1]
            )
            nc.vector.tensor_scalar_add(
                kt_sb[:, p * PW:(p + 1) * PW], pqk[HS:2 * HS, :],
                bcol_sb[HS:2 * HS, 0:1],
            )

        def vp_finish(p, pv):
            """V bias add (DVE) + V' via ONE blocked xbar DMA-transpose on
            the sync ring -- zero PE cost, single issue slot."""
            vt = vt_pool.tile([VPAD, PW], BF16, tag="vt", name=f"vt{p}")
            nc.gpsimd.memset(vt[HS:VPAD, :], 0.0)
            nc.vector.tensor_scalar_add(vt[0:VP, :], pv[0:VP, :], bcol_sb[0:VP, 1:2])
            nc.sync.dma_start_transpose(
                vp_sb[:, 8 * p * VPAD:(8 * p + 8) * VPAD].rearrange(
                    "q (c n) -> q c n", c=8
                ),
                vt[0:VPAD, :],
            )

        def attn(p, order, pre=None, filler=None):
            """Attention for pair p, k-tiles processed in `order` (narrow
            tiles interleaved among wide ones keeps the PE/exp pipeline
            uniformly PE-bound -- no chain-bound tail windows for the HAM to
            re-throttle on).  `pre` emits this pair's V'-transpose block
            after the first two S^T tiles (bridging the vt-add latency);
            `filler` yields next-pair projection chunks as PE filler.
            Returns deferred store closures for halves not stored inline."""
            a, b = 2 * p, 2 * p + 1
            nka, nk = 4 * a + 4, 4 * b + 4
            a_vis = [i for i, j in enumerate(order) if j < nka]
            a_first, a_last = min(a_vis), max(a_vis)
            ot = proj_t(f"ot{p}", [VP, PW])
            ptiles = {}

            def emit_s(j):
                wide = j < nka
                sp = sp_t(f"sp{p}_{j}")
                ksl = kt_sb[:, j * TT:(j + 1) * TT]
                if wide:
                    nc.tensor.matmul(
                        sp[:, 0:QS], ksl, qt_sb[:, a * QS:(a + 1) * QS],
                        start=True, stop=True,
                    )
                nc.tensor.matmul(
                    sp[:, QS:PW], ksl, qt_sb[:, b * QS:(b + 1) * QS],
                    start=True, stop=True,
                )
                pt = pt_pool.tile([TT, PW], BF16, tag="pt", name=f"pt{p}_{j}")
                if wide:
                    nc.scalar.activation(
                        pt[:], sp[:], mybir.ActivationFunctionType.Exp
                    )
                else:
                    nc.scalar.activation(
                        pt[:, QS:PW], sp[:, QS:PW],
                        mybir.ActivationFunctionType.Exp,
                    )
                for half, s in ((0, a), (1, b)):
                    dd = j - 4 * s
                    if 0 <= dd < 4:
                        nc.gpsimd.affine_select(
                            out=pt[:, half * QS:(half + 1) * QS],
                            in_=pt[:, half * QS:(half + 1) * QS],
                            compare_op=mybir.AluOpType.is_ge,
                            fill=0.0,
                            base=-TT * dd,
                            channel_multiplier=-1,
                            pattern=[[1, QS]],
                        )
                ptiles[j] = pt

            def emit_pv(idx, j):
                pt = ptiles.pop(j)
                vsl = vp_sb[:, j * VPAD:j * VPAD + VP]
                if j < nka:
                    nc.tensor.matmul(
                        ot[:, 0:QS], vsl, pt[:, 0:QS],
                        start=(idx == a_first), stop=(idx == a_last),
                        skip_group_check=True,
                    )
                nc.tensor.matmul(
                    ot[:, QS:PW], vsl, pt[:, QS:PW],
                    start=(idx == 0), stop=(idx == nk - 1),
                    skip_group_check=True,
                )

            def store_half(half):
                osb = osb_pool.tile([VP, QS], F32, tag="osb",
                                    name=f"osb{p}_{half}")
                nc.vector.tensor_copy(osb[:], ot[:, half * QS:(half + 1) * QS])
                nc.sync.dma_start(
                    out_ext[:, (2 * p + half) * QS:(2 * p + half + 1) * QS],
                    osb[:],
                )

            if filler is not None:
                # dep-free PE work ahead of the first S^T weight loads,
                # which gate on the DVE qt/kt drain chain
                next(filler, None)
                next(filler, None)
            if pre is not None:
                # vt-gated transposes go first; the PE reorder window lets
                # the S^T matmuls behind them start as soon as qt/kt land
                pre()
            emit_s(order[0])
            emit_s(order[1])
            deferred = []
            for idx in range(nk):
                if filler is not None:
                    # front-load the filler so it is exhausted before the
                    # exp-gated tail, and its trailing drain runs mid-phase
                    take = 2 if idx < 4 else 1
                    for _ in range(take):
                        next(filler, None)
                if idx + 2 < nk:
                    emit_s(order[idx + 2])
                emit_pv(idx, order[idx])
                if idx == a_last and idx < nk - 1:
                    store_half(0)
            if a_last == nk - 1:
                deferred.append(lambda: store_half(0))
            deferred.append(lambda: store_half(1))
            return deferred

        # ---- schedule: one dense PE stream ----
        # A01 consumes chunks in DMA-arrival order: evens (scalar ring)
        # land before odds (sync ring, queued behind W).
        pqk0 = proj_t("pqk0")
        for _ in proj_half(xt_sb, wqk_sb, pqk0,
                           chunk_order=[0, 2, 4, 1, 6, 3, 5, 7]):
            pass
        drain_qk(0, pqk0)

        pv0 = proj_t("pv0")
        for _ in proj_half(xt_sb, wv_sb, pv0):
            pass

        xt1 = [xt1_sb[:, c * PW:(c + 1) * PW] for c in range(NDT)]

        def filler0():
            pqk1 = proj_t("pqk1")
            yield from proj_half(xt1, wqk_sb, pqk1)
            drain_qk(1, pqk1)   # on DVE mid-attention, off the corridor
            yield

        def pre0():
            # dep-free pad while the vt-add drains on DVE
            for _ in range(2):
                nc.tensor.matmul(
                    warm_ps[:], warm_sb[:, 0:TT], warm_sb[:],
                    start=True, stop=True,
                )
            vp_finish(0, pv0)

        # pair0: every k-tile is diagonal in one half; alternate wide/narrow
        # so the stream stays PE-bound end to end.
        stores0 = attn(
            0,
            order=[0, 4, 1, 5, 2, 6, 3, 7],
            pre=pre0,
            filler=filler0(),
        )
        for s in stores0:   # overlapped by attn1's first S^T matmuls
            s()

        def filler1():
            """pair1 V projection as attn1 PE filler; V' finishes off-PE
            (DVE bias add + sync-ring DMA-transposes)."""
            pv1 = proj_t("pv1")
            yield from proj_half(xt1, wv_sb, pv1)
            vp_finish(1, pv1)
            yield

        # pair1: first tiles use pair0's V' (ready) while the filler builds
        # pair1's; a-half finishes at tile 11 (pos 14) so its store overlaps
        # the final narrow tile.
        stores1 = attn(
            1,
            order=[0, 1, 2, 3, 4, 5, 6, 7, 12, 8, 13, 9, 14, 10, 11, 15],
            filler=filler1(),
        )
        for s in stores1:
            s()

    nc.compile()
    return nc


def make_inputs(x_b, Wq, bq, Wk, bk, Wv, bv):
    """Host-side prep for one core's in_map (x_b: [T, D] fp32)."""
    import ml_dtypes

    bf = ml_dtypes.bfloat16
    scale = 1.0 / np.sqrt(np.float32(HS))
    wqk = np.zeros((D, TT), dtype=np.float32)
    wqk[:, 0:HS] = Wq * scale
    wqk[:, HS:2 * HS] = Wk
    wv = np.zeros((D, TT), dtype=np.float32)
    wv[:, 0:HS] = Wv

    def chunk_major(w):
        # w2[p, c*128 + n] = w[c*128 + p, n] -> contiguous [128, 1024] DMA
        return np.ascontiguousarray(
            w.reshape(NDT, TT, TT).transpose(1, 0, 2).reshape(TT, NDT * TT)
        )

    bcol = np.zeros((TT, 2), dtype=np.float32)
    bcol[0:HS, 0] = bq * scale
    bcol[HS:2 * HS, 0] = bk
    bcol[0:HS, 1] = bv
    bcol[HS, 1] = 1.0
    return {
        "xt": np.ascontiguousarray(x_b.T).astype(bf),
        "wqk": chunk_major(wqk).astype(bf),
        "wv": chunk_major(wv).astype(bf),
        "bcol": bcol,
    }


def finish_output(outT):
    """Host-side normalize + transpose: outT [65, T] -> [T, HS]."""
    o = np.asarray(outT, dtype=np.float32)
    return (o[0:HS, :] / o[HS:HS + 1, :]).T


_NC_CACHE = None


def _get_nc():
    global _NC_CACHE
    if _NC_CACHE is None:
        _NC_CACHE = build_graph()
    return _NC_CACHE


def kernel(x, Wq, bq, Wk, bk, Wv, bv):
    x = np.asarray(x, dtype=np.float32)
    args = [np.asarray(a, dtype=np.float32) for a in (Wq, bq, Wk, bk, Wv, bv)]
    nc = _get_nc()
    in_maps = [make_inputs(x[b], *args) for b in range(N_CORES)]
    trace = os.environ.get("BASS_ATTN_TRACE", "0") == "1"
    res = run_bass_kernel_spmd(
        nc, in_maps, core_ids=list(range(N_CORES)), trace=trace
    )
    if trace:
        print(
            f"HW exec time: {res.exec_time_ns} ns "
            f"(mean {res.mean_exec_time_ns}, max core {res.max_exec_time_core_id})"
        )
    out = np.stack(
        [finish_output(res.results[b]["outT"]) for b in range(N_CORES)], axis=0
    )
    return out
